# revision 60
# baseline (speedup 1.0000x reference)
"""GQA (grouped-query attention) Trainium2 Bass kernel, v3.

Problem: B=4, T=2048, E=1536, 8 kv-groups; per group one attention head of
dim D=192 (q projected to 192; k/v projected to 64 and channel-tiled 3x),
interleaved-pair RoPE on q and tiled-k, causal softmax, out = P @ v_tiled.

Structure (per core: one batch, 4 groups, two 2-group passes):
  * Host pre-transposes x to xT [E, T] in bf16 -> projection lhsT tiles
    DMA directly; no PE transposes for x.  All matmuls bf16.
  * RoPE elementwise bf16 (DVE 2x) on natt copies; q/k transposes bf16.
  * S^T layout; off-diagonal 512-blocks in fp8-e4m3 with DoubleRow
    (0.5 cycles/row); diagonal 512-blocks bf16 with N trimmed to the
    causally valid q range.  PV bf16, N trimmed the same way.
  * v not roped: P @ [v64 | ones]; ones col is the softmax denominator;
    output replicated 3x by a stride-0 DMA.  No max subtraction.
  * Software pipeline: pass h projects groups (2h, 2h+1); SDPA blocks of
    the previous pass's groups are emitted interleaved with proj tiles so
    Act-bound exp overlaps PE-bound projection.

Sharding: 8 cores = 4 batches x 2 group-halves; core writes (T, 768).
"""

import math
from contextlib import ExitStack

import numpy as np
import ml_dtypes

import concourse.bass as bass
import concourse.mybir as mybir
import concourse.tile as tile
from concourse import bacc
from concourse.bass_utils import run_bass_kernel_spmd
from concourse.masks import make_identity

B, T, E = 4, 2048, 1536
G = 8            # kv heads (groups)
HD = 64          # per-head dim of k/v before tiling
REP = 3
D = REP * HD     # 192, per-group attention dim
P = 128
NT = T // P      # 16 row tiles
NE = E // P      # 12 contraction chunks
GPC = 4          # groups per core
WCOLS = GPC * D + 2 * GPC * HD         # 1280
THETA = 10000.0
SCALE = 1.0 / math.sqrt(D)
QCH = 512        # q chunk (matmul free dim / PSUM bank)
NQC = T // QCH   # 4
DIAG = QCH // P  # 4 k-tiles per diagonal 512-region

F32 = mybir.dt.float32
BF16 = mybir.dt.bfloat16
FP8 = mybir.dt.float8e4
DR = mybir.MatmulPerfMode.DoubleRow


def _build_nc(use_bias=False, use_fp8=True):
    nc = bacc.Bacc("TRN2", target_bir_lowering=False, debug=False)

    xt_d = nc.dram_tensor("xt", [E, T], BF16, kind="ExternalInput").ap()
    w_d = nc.dram_tensor("w", [E, WCOLS], BF16, kind="ExternalInput").ap()
    b_d = nc.dram_tensor("bias", [1, WCOLS], BF16, kind="ExternalInput").ap()
    cos_d = nc.dram_tensor("cos", [T, D // 2], BF16, kind="ExternalInput").ap()
    sin_d = nc.dram_tensor("sin", [T, D // 2], BF16, kind="ExternalInput").ap()
    # fp8 3-term projection operands (hi16 / hi / lo16 of x; hi / lo of 32W)
    # x streams are host-tiled to [p, ti, (pl ep tt)] so the per-ti DMA
    # slice is a contiguous 1536B run per partition.
    x8_d = nc.dram_tensor("x8", [P, NT * 3 * NE * P], FP8,
                          kind="ExternalInput").ap()
    w8h_d = nc.dram_tensor("w8h", [E, WCOLS], FP8, kind="ExternalInput").ap()
    w8l_d = nc.dram_tensor("w8l", [E, WCOLS], FP8, kind="ExternalInput").ap()
    z8_d = nc.dram_tensor("z8", [D - P, GPC * T], FP8,
                          kind="ExternalInput").ap()
    out_d = nc.dram_tensor("out", [T, GPC * D], F32, kind="ExternalOutput").ap()

    mult = mybir.AluOpType.mult

    with tile.TileContext(nc) as tc, ExitStack() as ctx:
        singles = ctx.enter_context(tc.tile_pool(name="singles", bufs=1))
        qkv_pool = ctx.enter_context(tc.tile_pool(name="qkv", bufs=1))
        stream = ctx.enter_context(tc.tile_pool(name="stream", bufs=4))
        natp = ctx.enter_context(tc.tile_pool(name="natp", bufs=3))
        small = ctx.enter_context(tc.tile_pool(name="small", bufs=3))
        ppool = ctx.enter_context(tc.tile_pool(name="ppool", bufs=10))
        opool = ctx.enter_context(tc.tile_pool(name="opool", bufs=3))
        ps_proj = ctx.enter_context(tc.tile_pool(name="ps_proj", bufs=1, space="PSUM"))
        ps_t = ctx.enter_context(tc.tile_pool(name="ps_t", bufs=2, space="PSUM"))
        ps_s = ctx.enter_context(tc.tile_pool(name="ps_s", bufs=2, space="PSUM"))
        ps_o = ctx.enter_context(tc.tile_pool(name="ps_o", bufs=1, space="PSUM"))

        ident = singles.tile([P, P], BF16)
        make_identity(nc, ident)
        ident_f = singles.tile([P, P], F32)
        make_identity(nc, ident_f)
        ones = singles.tile([1, P], BF16)
        nc.vector.memset(ones, 1.0)
        # causal triangle mask: tri[p, f] = 1.0 if f >= p else 0
        tri = singles.tile([P, P], BF16, name="tri", tag="tri")
        nc.gpsimd.memset(tri, 1.0)
        nc.gpsimd.affine_select(
            out=tri, in_=tri, pattern=[[1, P]],
            compare_op=mybir.AluOpType.is_ge, fill=0.0,
            base=0, channel_multiplier=-1)

        use_fp8proj = not use_bias
        if use_fp8proj:
            w8h = singles.tile([P, 2, NE // 2, WCOLS], FP8)
            w8l = singles.tile([P, 2, NE // 2, WCOLS], FP8)
            w8h_r = w8h_d.rearrange("(ep pl p) c -> p pl ep c", pl=2, p=P)
            w8l_r = w8l_d.rearrange("(ep pl p) c -> p pl ep c", pl=2, p=P)
            for ep in range(NE // 2):
                nc.sync.dma_start(w8h[:, :, ep, :], w8h_r[:, :, ep, :])
                nc.scalar.dma_start(w8l[:, :, ep, :], w8l_r[:, :, ep, :])
        else:
            w_sb = singles.tile([P, NE, WCOLS], BF16)
            w_r = w_d.rearrange("(eo p) c -> p eo c", p=P)
            for eo in range(NE):
                nc.sync.dma_start(w_sb[:, eo, :], w_r[:, eo, :])
        b_sb = singles.tile([1, WCOLS], BF16)
        nc.sync.dma_start(b_sb, b_d)
        cos_sb = singles.tile([P, NT, D // 2], BF16)
        nc.sync.dma_start(cos_sb, cos_d.rearrange("(n p) c -> p n c", p=P))
        sin_sb = singles.tile([P, NT, D // 2], BF16)
        nc.sync.dma_start(sin_sb, sin_d.rearrange("(n p) c -> p n c", p=P))

        # persistent q/k/v storage, all 4 groups
        qT_hi = qkv_pool.tile([P, GPC, T], BF16, tag="qT_hi", name="qT_hi")
        qT_lo = qkv_pool.tile([D - P, GPC, T], BF16, tag="qT_lo",
                              name="qT_lo")
        kT_hi = qkv_pool.tile([P, GPC, T], BF16, tag="kT_hi", name="kT_hi")
        kT_lo = qkv_pool.tile([D - P, GPC, T], BF16, tag="kT_lo",
                              name="kT_lo")
        v_sb = qkv_pool.tile([P, NT, GPC, HD + 1], BF16, tag="v_sb",
                             name="v_sb")
        nc.gpsimd.memset(v_sb[:, :, :, HD:HD + 1], 1.0)
        if use_fp8:
            q8 = qkv_pool.tile([P, 2, GPC, T], FP8, tag="q8", name="q8")
            k8 = qkv_pool.tile([P, 2, GPC, T], FP8, tag="k8", name="k8")
            # pad-row zeroing via DMA so no engine queue is blocked at start
            zr = z8_d.rearrange("p (g t) -> p g t", g=GPC)
            nc.sync.dma_start(q8[D - P:P, 1], zr)
            nc.sync.dma_start(k8[D - P:P, 1], zr)

        def emit_rope(ti, natt):
            cosv = cos_sb[:, ti, :]
            sinv = sin_sb[:, ti, :]
            # --- q rope, all groups at once (rotate-half layout) ---
            qv = natt[:, 0:GPC * D].rearrange("p (g d) -> p g d", g=GPC)
            qR = qv[:, :, 0:D // 2]
            qI = qv[:, :, D // 2:D]
            cosb = cosv[:, None, :].to_broadcast((P, GPC, D // 2))
            sinb = sinv[:, None, :].to_broadcast((P, GPC, D // 2))
            qrot = small.tile([P, GPC * D], BF16, tag="qrot", name="qrot")
            qo = qrot.rearrange("p (g d) -> p g d", g=GPC)
            qo0 = qo[:, :, 0:D // 2]
            qo1 = qo[:, :, D // 2:D]
            tmp = small.tile([P, GPC * (D // 2)], BF16, tag="ropetmp",
                             name="ropetmp")
            tmpg = tmp.rearrange("p (g d) -> p g d", g=GPC)
            nc.vector.tensor_tensor(qo0, qR, cosb, mult)
            nc.vector.tensor_tensor(tmpg, qI, sinb, mult)
            nc.vector.tensor_sub(qo0, qo0, tmpg)
            nc.vector.tensor_tensor(qo1, qR, sinb, mult)
            nc.vector.tensor_tensor(tmpg, qI, cosb, mult)
            nc.vector.tensor_add(qo1, qo1, tmpg)

            # --- k: expand 64 -> 192 with per-copy rope ---
            kv = natt[:, GPC * D:GPC * D + GPC * HD].rearrange(
                "p (g c) -> p g c", g=GPC)
            kR = kv[:, :, None, 0:32].to_broadcast((P, GPC, REP, 32))
            kI = kv[:, :, None, 32:HD].to_broadcast((P, GPC, REP, 32))
            cos3 = cosv.rearrange("p (r c) -> p r c", r=REP)
            sin3 = sinv.rearrange("p (r c) -> p r c", r=REP)
            cos3b = cos3[:, None, :, :].to_broadcast((P, GPC, REP, 32))
            sin3b = sin3[:, None, :, :].to_broadcast((P, GPC, REP, 32))
            krot = small.tile([P, GPC * D], BF16, tag="krot", name="krot")
            ko = krot.rearrange("p (g u r c) -> p g u r c", g=GPC, u=2, r=REP)
            ko0 = ko[:, :, 0]
            ko1 = ko[:, :, 1]
            tmp3 = tmpg.rearrange("p g (r c) -> p g r c", r=REP)
            nc.vector.tensor_tensor(ko0, kR, cos3b, mult)
            nc.vector.tensor_tensor(tmp3, kI, sin3b, mult)
            nc.vector.tensor_sub(ko0, ko0, tmp3)
            nc.vector.tensor_tensor(ko1, kR, sin3b, mult)
            nc.vector.tensor_tensor(tmp3, kI, cos3b, mult)
            nc.vector.tensor_add(ko1, ko1, tmp3)

            # --- transposes into PSUM (bf16) ---
            tq_hi = ps_t.tile([P, GPC * P], BF16, tag="tps", name="tq_hi")
            tq_lo = ps_t.tile([D - P, GPC * P], BF16, tag="tps", name="tq_lo")
            for g in range(GPC):
                nc.tensor.transpose(tq_hi[:, g * P:(g + 1) * P],
                                    qrot[:, g * D:g * D + P], ident)
                nc.tensor.transpose(tq_lo[:, g * P:(g + 1) * P],
                                    qrot[:, g * D + P:(g + 1) * D], ident)
            qhi_d = qT_hi[:, :, ti * P:(ti + 1) * P]
            qlo_d = qT_lo[:, :, ti * P:(ti + 1) * P]
            nc.vector.tensor_copy(qhi_d,
                                  tq_hi.rearrange("p (g t) -> p g t", g=GPC))
            nc.scalar.copy(qlo_d,
                           tq_lo.rearrange("p (g t) -> p g t", g=GPC))
            if use_fp8:
                # fp8 copies read the SBUF bf16 qT (Pool cannot touch PSUM)
                nc.gpsimd.tensor_copy(q8[:, 0, :, ti * P:(ti + 1) * P],
                                      qhi_d)
                nc.gpsimd.tensor_copy(q8[0:D - P, 1, :, ti * P:(ti + 1) * P],
                                      qlo_d)
            tk_hi = ps_t.tile([P, GPC * P], BF16, tag="tps", name="tk_hi")
            tk_lo = ps_t.tile([D - P, GPC * P], BF16, tag="tps", name="tk_lo")
            for g in range(GPC):
                nc.tensor.transpose(tk_hi[:, g * P:(g + 1) * P],
                                    krot[:, g * D:g * D + P], ident)
                nc.tensor.transpose(tk_lo[:, g * P:(g + 1) * P],
                                    krot[:, g * D + P:(g + 1) * D], ident)
            khi_d = kT_hi[:, :, ti * P:(ti + 1) * P]
            klo_d = kT_lo[:, :, ti * P:(ti + 1) * P]
            nc.vector.tensor_copy(khi_d,
                                  tk_hi.rearrange("p (g t) -> p g t", g=GPC))
            nc.scalar.copy(klo_d,
                           tk_lo.rearrange("p (g t) -> p g t", g=GPC))
            if use_fp8:
                nc.gpsimd.tensor_copy(k8[:, 0, :, ti * P:(ti + 1) * P],
                                      khi_d)
                nc.gpsimd.tensor_copy(k8[0:D - P, 1, :, ti * P:(ti + 1) * P],
                                      klo_d)

            # --- v copy (col HD is the ones column); SBUF-only -> Pool ---
            vb = GPC * D + GPC * HD
            nc.gpsimd.tensor_copy(
                v_sb[:, ti, :, 0:HD],
                natt[:, vb:vb + GPC * HD].rearrange("p (g c) -> p g c", g=GPC))

        COPYF = mybir.ActivationFunctionType.Copy

        def emit_proj_tile(ti):
            half = GPC // 2 * D  # 384
            natt = natp.tile([P, WCOLS], BF16, tag="natt", name="natt")
            pq_a = ps_proj.tile([P, half], F32, tag="pq_a", name="pq_a")
            pq_b = ps_proj.tile([P, half], F32, tag="pq_b", name="pq_b")
            pkv = ps_proj.tile([P, 2 * GPC * HD], F32, tag="pkv", name="pkv")
            if use_fp8proj:
                # P = 16*xh@Wh + xh@Wl + xl16@Wh = 512 * x@W (+O(1e-4))
                x8t = stream.tile([P, 3, 2, NE // 2, P], FP8, tag="x8t",
                                  name="x8t")
                nrow = 3 * NE * P  # 4608 contiguous elems/partition per ti
                nc.gpsimd.dma_start(
                    x8t.rearrange("p v pl ep t -> p (v pl ep t)"),
                    x8_d[:, ti * nrow:(ti + 1) * nrow])
                terms = ((x8t[:, 0], w8h), (x8t[:, 1], w8l),
                         (x8t[:, 2], w8h))
                for ep in range(NE // 2):
                    for t, (xv, wv) in enumerate(terms):
                        lhsT = xv[:, :, ep, :]
                        first = (ep == 0 and t == 0)
                        last = (ep == NE // 2 - 1 and t == 2)
                        nc.tensor.matmul(
                            pq_a, lhsT, wv[:, :, ep, 0:half],
                            start=first, stop=last, perf_mode=DR)
                        nc.tensor.matmul(
                            pq_b, lhsT, wv[:, :, ep, half:2 * half],
                            start=first, stop=last, perf_mode=DR)
                        nc.tensor.matmul(
                            pkv, lhsT, wv[:, :, ep, GPC * D:WCOLS],
                            start=first, stop=last, perf_mode=DR)
                sc = 1.0 / 512.0
                nc.scalar.activation(natt[:, 0:half], pq_a, COPYF, scale=sc)
                nc.scalar.activation(natt[:, half:2 * half], pq_b, COPYF,
                                     scale=sc)
                nc.scalar.activation(natt[:, GPC * D:WCOLS], pkv, COPYF,
                                     scale=sc)
                return natt
            xti = stream.tile([P, NE, P], BF16, tag="xti", name="xti")
            nc.gpsimd.dma_start(
                xti, xt_d[:, ti * P:(ti + 1) * P].rearrange(
                    "(eo p) t -> p eo t", p=P))
            for eo in range(NE):
                lhsT = xti[:, eo, :]
                last = (eo == NE - 1) and not use_bias
                nc.tensor.matmul(
                    pq_a, lhsT, w_sb[:, eo, 0:half],
                    start=(eo == 0), stop=last)
                nc.tensor.matmul(
                    pq_b, lhsT, w_sb[:, eo, half:2 * half],
                    start=(eo == 0), stop=last)
                nc.tensor.matmul(
                    pkv, lhsT, w_sb[:, eo, GPC * D:WCOLS],
                    start=(eo == 0), stop=last)
            if use_bias:
                nc.tensor.matmul(pq_a, ones, b_sb[:, 0:half],
                                 start=False, stop=True)
                nc.tensor.matmul(pq_b, ones, b_sb[:, half:2 * half],
                                 start=False, stop=True)
                nc.tensor.matmul(pkv, ones, b_sb[:, GPC * D:WCOLS],
                                 start=False, stop=True)
            nc.scalar.copy(natt[:, 0:half], pq_a)
            nc.scalar.copy(natt[:, half:2 * half], pq_b)
            nc.vector.tensor_copy(natt[:, GPC * D:WCOLS], pkv)
            return natt

        # SDPA sub-chunks: (qoff, width).  qc0..2 are full 512 chunks; the
        # last 512 is split in two 256 halves so the first becomes ready
        # two proj tiles earlier (shrinks the post-proj tail).
        CHUNKS = [(qc * QCH, QCH) for qc in range(NQC)]
        BLOCKS = [(j, qo, w, kc) for (qo, w) in CHUNKS for j in range(GPC)
                  for kc in range((qo + w) // P)]
        LA = 8

        def sdpa_steps():
            """Generator: one SDPA block per next().  Groups interleave at
            sub-chunk granularity so only one (group, chunk) owns the
            o_ps/tpo rings at a time."""

            def emit_s(j, qo, w, kc):
                diag = kc * P >= qo
                off = max(kc * P - qo, 0)  # chunk-frame offset
                nq = w - off
                s_ps = ps_s.tile([P, QCH], F32, tag="sps", name="sps")
                sv = s_ps[:, off:w]
                if use_fp8 and (kc + 1) * P <= (qo // QCH) * QCH:
                    nc.tensor.matmul(
                        sv, k8[:, :, j, kc * P:(kc + 1) * P],
                        q8[:, :, j, qo + off:qo + off + nq],
                        start=True, stop=True, perf_mode=DR)
                else:
                    nc.tensor.matmul(
                        sv, kT_hi[:, j, kc * P:(kc + 1) * P],
                        qT_hi[:, j, qo + off:qo + off + nq],
                        start=True, stop=False)
                    nc.tensor.matmul(
                        sv, kT_lo[:, j, kc * P:(kc + 1) * P],
                        qT_lo[:, j, qo + off:qo + off + nq],
                        start=False, stop=True)
                pT = ppool.tile([P, QCH], BF16, tag="pT", name="pT")
                nc.scalar.activation(pT[:, off:w], sv,
                                     mybir.ActivationFunctionType.Exp,
                                     scale=SCALE)
                if diag:  # causal zeroing of the in-block triangle
                    nc.gpsimd.tensor_tensor(pT[:, off:off + P],
                                            pT[:, off:off + P],
                                            tri, mult)
                return pT

            pTs = {}
            for i in range(LA):
                pTs[BLOCKS[i]] = emit_s(*BLOCKS[i])
            o_ps = None
            for i, (j, qo, w, kc) in enumerate(BLOCKS):
                if i + LA < len(BLOCKS):
                    b = BLOCKS[i + LA]
                    pTs[b] = emit_s(*b)
                kmax = (qo + w) // P
                if kc == 0:
                    o_ps = ps_o.tile([HD + 1, QCH], F32, tag="ops",
                                     name="ops")
                pw = pTs.pop((j, qo, w, kc))
                off = max(kc * P - qo, 0)
                nc.tensor.matmul(o_ps[:, off:w], v_sb[:, kc, j, :],
                                 pw[:, off:w],
                                 start=(kc == 0), stop=(kc == kmax - 1))
                yield
                if kc != kmax - 1:
                    continue
                # ---- finalize (group j, chunk qo..qo+w) ----
                o_sb = opool.tile([HD + 1, QCH], F32, tag="o_sb",
                                  name="o_sb")
                nc.vector.tensor_copy(o_sb[:, 0:w], o_ps[:, 0:w])
                NB = w // P
                tpo = ps_o.tile([P, (QCH // P) * (HD + 1)], F32, tag="ops",
                                name="tpo")
                for blk in range(NB):
                    nc.tensor.transpose(
                        tpo[:, blk * (HD + 1):(blk + 1) * (HD + 1)],
                        o_sb[:, blk * P:(blk + 1) * P],
                        ident_f[:HD + 1, :HD + 1])
                nat = opool.tile([P, QCH // P, HD + 8], F32, tag="nat",
                                 name="nat")
                nc.vector.tensor_copy(
                    nat[:, 0:NB, 0:HD + 1],
                    tpo[:, 0:NB * (HD + 1)].rearrange(
                        "p (b c) -> p b c", b=NB))
                rec = opool.tile([P, QCH // P], F32, tag="rec", name="rec")
                nc.vector.reciprocal(rec[:, 0:NB], nat[:, 0:NB, HD])
                nc.vector.tensor_tensor(
                    nat[:, 0:NB, 0:HD], nat[:, 0:NB, 0:HD],
                    rec[:, 0:NB, None].to_broadcast((P, NB, HD)), mult)
                for blk in range(NB):
                    row0 = qo + blk * P
                    dst = out_d[row0:row0 + P,
                                j * D:(j + 1) * D].rearrange(
                        "t (r c) -> t r c", r=REP)
                    src_ap = nat[:, blk, None, 0:HD].to_broadcast(
                        (P, REP, HD))
                    nc.sync.dma_start(dst, src_ap)
                yield

        # Per-yield readiness: number of roped proj tiles required.
        def ready(b):
            _, qo, w, _ = b
            return (qo + w) // P

        reqs = []
        for i, b in enumerate(BLOCKS):
            j, qo, w, kc = b
            reqs.append(ready(BLOCKS[min(i + LA, len(BLOCKS) - 1)]))
            if kc == (qo + w) // P - 1:
                reqs.append(ready(b))

        gen = sdpa_steps()
        state = {"idx": 0}

        def drain(ti_done, budget):
            while budget > 0 and state["idx"] < len(reqs):
                if reqs[state["idx"]] > ti_done:
                    return
                next(gen)
                state["idx"] += 1
                budget -= 1

        pending = []
        for ti in range(NT):
            natt = emit_proj_tile(ti)
            pending.append((ti, natt))
            if len(pending) > 1:
                emit_rope(*pending.pop(0))
            drain(ti, 24)
        while pending:
            emit_rope(*pending.pop(0))
        drain(NT, 1 << 30)

    nc.compile()
    return nc


_NC_CACHE = {}


def _get_nc(use_bias=True):
    if use_bias not in _NC_CACHE:
        _NC_CACHE[use_bias] = _build_nc(use_bias)
    return _NC_CACHE[use_bias]


def _host_inputs(x, Wq, bq, Wk, bk, Wv, bv):
    j = np.arange(D // 2)
    angles = 1.0 / (THETA ** ((2.0 * j) / D))
    th = np.arange(T, dtype=np.float64)[:, None] * angles[None, :]
    cosn = np.cos(th).astype(ml_dtypes.bfloat16)
    sinn = np.sin(th).astype(ml_dtypes.bfloat16)

    perm_q = np.concatenate([np.arange(0, D, 2), np.arange(1, D, 2)])
    eo = np.concatenate([np.arange(0, HD, 2), np.arange(1, HD, 2)])

    Wq = np.asarray(Wq, np.float32)
    Wk = np.asarray(Wk, np.float32)
    Wv = np.asarray(Wv, np.float32)
    bq = np.asarray(bq, np.float32)
    bk = np.asarray(bk, np.float32)
    bv = np.asarray(bv, np.float32)
    x = np.asarray(x, np.float32)

    in_maps = []
    for c in range(8):
        b, gh = divmod(c, 2)
        gs = [gh * GPC + jj for jj in range(GPC)]
        wblocks, bblocks = [], []
        for g in gs:
            wblocks.append(Wq[:, g * D:(g + 1) * D][:, perm_q])
            bblocks.append(bq[g * D:(g + 1) * D][perm_q])
        for g in gs:
            wblocks.append(Wk[:, g * HD:(g + 1) * HD][:, eo])
            bblocks.append(bk[g * HD:(g + 1) * HD][eo])
        for g in gs:
            wblocks.append(Wv[:, g * HD:(g + 1) * HD])
            bblocks.append(bv[g * HD:(g + 1) * HD])
        w_core = np.ascontiguousarray(np.concatenate(wblocks, axis=1))
        b_core = np.concatenate(bblocks)[None, :].astype(ml_dtypes.bfloat16)
        b_core = np.ascontiguousarray(b_core)
        xt = np.ascontiguousarray(x[b].T)
        f8 = ml_dtypes.float8_e4m3
        xh = xt.astype(f8).astype(np.float32)
        w32 = 32.0 * w_core
        wh = w32.astype(f8).astype(np.float32)

        def xtile(a):
            # [E, T] -> [p, ti, pl, ep, tt]
            a = a.reshape(NE // 2, 2, P, NT, P)       # ep pl p ti tt
            return a.transpose(2, 3, 1, 0, 4)         # p ti pl ep tt

        x8 = np.stack([xtile((16.0 * xh).astype(f8)),
                       xtile(xh.astype(f8)),
                       xtile((16.0 * (xt - xh)).astype(f8))],
                      axis=2)                         # p ti v pl ep tt
        x8 = np.ascontiguousarray(x8.reshape(P, NT * 3 * NE * P))

        in_maps.append({
            "xt": xt.astype(ml_dtypes.bfloat16),
            "w": w_core.astype(ml_dtypes.bfloat16),
            "bias": b_core,
            "cos": cosn,
            "sin": sinn,
            "x8": x8,
            "w8h": wh.astype(f8),
            "w8l": (16.0 * (w32 - wh)).astype(f8),
            "z8": np.zeros((D - P, GPC * T), f8),
        })
    return in_maps


def kernel(x, Wq, bq, Wk, bk, Wv, bv, _trace=False, _trace_kwargs=None):
    in_maps = _host_inputs(x, Wq, bq, Wk, bk, Wv, bv)
    use_bias = bool(max(np.abs(np.asarray(b)).max() for b in (bq, bk, bv)) > 0)
    nc = _get_nc(use_bias)
    res = run_bass_kernel_spmd(nc, in_maps, core_ids=list(range(8)),
                               trace=_trace, **(_trace_kwargs or {}))
    out = np.empty((B, T, E), np.float32)
    for c in range(8):
        b, gh = divmod(c, 2)
        out[b, :, gh * GPC * D:(gh + 1) * GPC * D] = res.results[c]["out"]
    if _trace:
        return out, res
    return out


# revision 61
# speedup vs baseline: 1.0160x; 1.0160x over previous
"""GQA (grouped-query attention) Trainium2 Bass kernel, v3.

Problem: B=4, T=2048, E=1536, 8 kv-groups; per group one attention head of
dim D=192 (q projected to 192; k/v projected to 64 and channel-tiled 3x),
interleaved-pair RoPE on q and tiled-k, causal softmax, out = P @ v_tiled.

Structure (per core: one batch, 4 groups, two 2-group passes):
  * Host pre-transposes x to xT [E, T] in bf16 -> projection lhsT tiles
    DMA directly; no PE transposes for x.  All matmuls bf16.
  * RoPE elementwise bf16 (DVE 2x) on natt copies; q/k transposes bf16.
  * S^T layout; off-diagonal 512-blocks in fp8-e4m3 with DoubleRow
    (0.5 cycles/row); diagonal 512-blocks bf16 with N trimmed to the
    causally valid q range.  PV bf16, N trimmed the same way.
  * v not roped: P @ [v64 | ones]; ones col is the softmax denominator;
    output replicated 3x by a stride-0 DMA.  No max subtraction.
  * Software pipeline: pass h projects groups (2h, 2h+1); SDPA blocks of
    the previous pass's groups are emitted interleaved with proj tiles so
    Act-bound exp overlaps PE-bound projection.

Sharding: 8 cores = 4 batches x 2 group-halves; core writes (T, 768).
"""

import math
from contextlib import ExitStack

import numpy as np
import ml_dtypes

import concourse.bass as bass
import concourse.mybir as mybir
import concourse.tile as tile
from concourse import bacc
from concourse.bass_utils import run_bass_kernel_spmd
from concourse.masks import make_identity

B, T, E = 4, 2048, 1536
G = 8            # kv heads (groups)
HD = 64          # per-head dim of k/v before tiling
REP = 3
D = REP * HD     # 192, per-group attention dim
P = 128
NT = T // P      # 16 row tiles
NE = E // P      # 12 contraction chunks
GPC = 4          # groups per core
WCOLS = GPC * D + 2 * GPC * HD         # 1280
THETA = 10000.0
SCALE = 1.0 / math.sqrt(D)
QCH = 512        # q chunk (matmul free dim / PSUM bank)
NQC = T // QCH   # 4
DIAG = QCH // P  # 4 k-tiles per diagonal 512-region

F32 = mybir.dt.float32
BF16 = mybir.dt.bfloat16
FP8 = mybir.dt.float8e4
DR = mybir.MatmulPerfMode.DoubleRow


def _build_nc(use_bias=False, use_fp8=True):
    nc = bacc.Bacc("TRN2", target_bir_lowering=False, debug=False)

    xt_d = nc.dram_tensor("xt", [E, T], BF16, kind="ExternalInput").ap()
    w_d = nc.dram_tensor("w", [E, WCOLS], BF16, kind="ExternalInput").ap()
    b_d = nc.dram_tensor("bias", [1, WCOLS], BF16, kind="ExternalInput").ap()
    cos_d = nc.dram_tensor("cos", [T, D // 2], BF16, kind="ExternalInput").ap()
    sin_d = nc.dram_tensor("sin", [T, D // 2], BF16, kind="ExternalInput").ap()
    # fp8 3-term projection operands (hi16 / hi / lo16 of x; hi / lo of 32W)
    # x streams are host-tiled to [p, ti, (pl ep tt)] so the per-ti DMA
    # slice is a contiguous 1536B run per partition.
    x8_d = nc.dram_tensor("x8", [P, NT * 3 * NE * P], FP8,
                          kind="ExternalInput").ap()
    w8h_d = nc.dram_tensor("w8h", [E, WCOLS], FP8, kind="ExternalInput").ap()
    w8l_d = nc.dram_tensor("w8l", [E, WCOLS], FP8, kind="ExternalInput").ap()
    z8_d = nc.dram_tensor("z8", [D - P, GPC * T], FP8,
                          kind="ExternalInput").ap()
    out_d = nc.dram_tensor("out", [T, GPC * D], F32, kind="ExternalOutput").ap()

    mult = mybir.AluOpType.mult

    with tile.TileContext(nc) as tc, ExitStack() as ctx:
        singles = ctx.enter_context(tc.tile_pool(name="singles", bufs=1))
        qkv_pool = ctx.enter_context(tc.tile_pool(name="qkv", bufs=1))
        stream = ctx.enter_context(tc.tile_pool(name="stream", bufs=4))
        natp = ctx.enter_context(tc.tile_pool(name="natp", bufs=3))
        small = ctx.enter_context(tc.tile_pool(name="small", bufs=3))
        ppool = ctx.enter_context(tc.tile_pool(name="ppool", bufs=10))
        opool = ctx.enter_context(tc.tile_pool(name="opool", bufs=3))
        ps_proj = ctx.enter_context(tc.tile_pool(name="ps_proj", bufs=1, space="PSUM"))
        ps_t = ctx.enter_context(tc.tile_pool(name="ps_t", bufs=2, space="PSUM"))
        ps_s = ctx.enter_context(tc.tile_pool(name="ps_s", bufs=2, space="PSUM"))
        ps_o = ctx.enter_context(tc.tile_pool(name="ps_o", bufs=1, space="PSUM"))

        ident = singles.tile([P, P], BF16)
        make_identity(nc, ident)
        ident_f = singles.tile([P, P], F32)
        make_identity(nc, ident_f)
        ones = singles.tile([1, P], BF16)
        nc.vector.memset(ones, 1.0)
        # causal triangle mask: tri[p, f] = 1.0 if f >= p else 0
        tri = singles.tile([P, P], BF16, name="tri", tag="tri")
        nc.gpsimd.memset(tri, 1.0)
        nc.gpsimd.affine_select(
            out=tri, in_=tri, pattern=[[1, P]],
            compare_op=mybir.AluOpType.is_ge, fill=0.0,
            base=0, channel_multiplier=-1)

        use_fp8proj = not use_bias
        if use_fp8proj:
            w8h = singles.tile([P, 2, NE // 2, WCOLS], FP8)
            w8l = singles.tile([P, 2, NE // 2, WCOLS], FP8)
            w8h_r = w8h_d.rearrange("(ep pl p) c -> p pl ep c", pl=2, p=P)
            w8l_r = w8l_d.rearrange("(ep pl p) c -> p pl ep c", pl=2, p=P)
            for ep in range(NE // 2):
                nc.sync.dma_start(w8h[:, :, ep, :], w8h_r[:, :, ep, :])
                nc.scalar.dma_start(w8l[:, :, ep, :], w8l_r[:, :, ep, :])
        else:
            w_sb = singles.tile([P, NE, WCOLS], BF16)
            w_r = w_d.rearrange("(eo p) c -> p eo c", p=P)
            for eo in range(NE):
                nc.sync.dma_start(w_sb[:, eo, :], w_r[:, eo, :])
        b_sb = singles.tile([1, WCOLS], BF16)
        nc.sync.dma_start(b_sb, b_d)
        cos_sb = singles.tile([P, NT, D // 2], BF16)
        nc.sync.dma_start(cos_sb, cos_d.rearrange("(n p) c -> p n c", p=P))
        sin_sb = singles.tile([P, NT, D // 2], BF16)
        nc.sync.dma_start(sin_sb, sin_d.rearrange("(n p) c -> p n c", p=P))

        # persistent q/k/v storage, all 4 groups
        qT_hi = qkv_pool.tile([P, GPC, T], BF16, tag="qT_hi", name="qT_hi")
        qT_lo = qkv_pool.tile([D - P, GPC, T], BF16, tag="qT_lo",
                              name="qT_lo")
        kT_hi = qkv_pool.tile([P, GPC, T], BF16, tag="kT_hi", name="kT_hi")
        kT_lo = qkv_pool.tile([D - P, GPC, T], BF16, tag="kT_lo",
                              name="kT_lo")
        v_sb = qkv_pool.tile([P, NT, GPC, HD + 1], BF16, tag="v_sb",
                             name="v_sb")
        nc.gpsimd.memset(v_sb[:, :, :, HD:HD + 1], 1.0)
        if use_fp8:
            q8 = qkv_pool.tile([P, 2, GPC, T], FP8, tag="q8", name="q8")
            k8 = qkv_pool.tile([P, 2, GPC, T], FP8, tag="k8", name="k8")
            # pad-row zeroing via DMA so no engine queue is blocked at start
            zr = z8_d.rearrange("p (g t) -> p g t", g=GPC)
            nc.sync.dma_start(q8[D - P:P, 1], zr)
            nc.sync.dma_start(k8[D - P:P, 1], zr)

        def emit_rope(ti, natt):
            cosv = cos_sb[:, ti, :]
            sinv = sin_sb[:, ti, :]
            # --- q rope, all groups at once (rotate-half layout) ---
            qv = natt[:, 0:GPC * D].rearrange("p (g d) -> p g d", g=GPC)
            qR = qv[:, :, 0:D // 2]
            qI = qv[:, :, D // 2:D]
            cosb = cosv[:, None, :].to_broadcast((P, GPC, D // 2))
            sinb = sinv[:, None, :].to_broadcast((P, GPC, D // 2))
            qrot = small.tile([P, GPC * D], BF16, tag="qrot", name="qrot")
            qo = qrot.rearrange("p (g d) -> p g d", g=GPC)
            qo0 = qo[:, :, 0:D // 2]
            qo1 = qo[:, :, D // 2:D]
            tmp = small.tile([P, GPC * (D // 2)], BF16, tag="ropetmp",
                             name="ropetmp")
            tmpg = tmp.rearrange("p (g d) -> p g d", g=GPC)
            nc.vector.tensor_tensor(qo0, qR, cosb, mult)
            nc.vector.tensor_tensor(tmpg, qI, sinb, mult)
            nc.vector.tensor_sub(qo0, qo0, tmpg)
            nc.vector.tensor_tensor(qo1, qR, sinb, mult)
            nc.vector.tensor_tensor(tmpg, qI, cosb, mult)
            nc.vector.tensor_add(qo1, qo1, tmpg)

            # --- k: expand 64 -> 192 with per-copy rope ---
            kv = natt[:, GPC * D:GPC * D + GPC * HD].rearrange(
                "p (g c) -> p g c", g=GPC)
            kR = kv[:, :, None, 0:32].to_broadcast((P, GPC, REP, 32))
            kI = kv[:, :, None, 32:HD].to_broadcast((P, GPC, REP, 32))
            cos3 = cosv.rearrange("p (r c) -> p r c", r=REP)
            sin3 = sinv.rearrange("p (r c) -> p r c", r=REP)
            cos3b = cos3[:, None, :, :].to_broadcast((P, GPC, REP, 32))
            sin3b = sin3[:, None, :, :].to_broadcast((P, GPC, REP, 32))
            krot = small.tile([P, GPC * D], BF16, tag="krot", name="krot")
            ko = krot.rearrange("p (g u r c) -> p g u r c", g=GPC, u=2, r=REP)
            ko0 = ko[:, :, 0]
            ko1 = ko[:, :, 1]
            tmp3 = tmpg.rearrange("p g (r c) -> p g r c", r=REP)
            nc.vector.tensor_tensor(ko0, kR, cos3b, mult)
            nc.vector.tensor_tensor(tmp3, kI, sin3b, mult)
            nc.vector.tensor_sub(ko0, ko0, tmp3)
            nc.vector.tensor_tensor(ko1, kR, sin3b, mult)
            nc.vector.tensor_tensor(tmp3, kI, cos3b, mult)
            nc.vector.tensor_add(ko1, ko1, tmp3)

            # --- transposes into PSUM (bf16) ---
            tq_hi = ps_t.tile([P, GPC * P], BF16, tag="tps", name="tq_hi")
            tq_lo = ps_t.tile([D - P, GPC * P], BF16, tag="tps", name="tq_lo")
            for g in range(GPC):
                nc.tensor.transpose(tq_hi[:, g * P:(g + 1) * P],
                                    qrot[:, g * D:g * D + P], ident)
                nc.tensor.transpose(tq_lo[:, g * P:(g + 1) * P],
                                    qrot[:, g * D + P:(g + 1) * D], ident)
            qhi_d = qT_hi[:, :, ti * P:(ti + 1) * P]
            qlo_d = qT_lo[:, :, ti * P:(ti + 1) * P]
            nc.vector.tensor_copy(qhi_d,
                                  tq_hi.rearrange("p (g t) -> p g t", g=GPC))
            nc.scalar.copy(qlo_d,
                           tq_lo.rearrange("p (g t) -> p g t", g=GPC))
            if use_fp8:
                # fp8 copies read the SBUF bf16 qT (Pool cannot touch PSUM)
                nc.gpsimd.tensor_copy(q8[:, 0, :, ti * P:(ti + 1) * P],
                                      qhi_d)
                nc.gpsimd.tensor_copy(q8[0:D - P, 1, :, ti * P:(ti + 1) * P],
                                      qlo_d)
            tk_hi = ps_t.tile([P, GPC * P], BF16, tag="tps", name="tk_hi")
            tk_lo = ps_t.tile([D - P, GPC * P], BF16, tag="tps", name="tk_lo")
            for g in range(GPC):
                nc.tensor.transpose(tk_hi[:, g * P:(g + 1) * P],
                                    krot[:, g * D:g * D + P], ident)
                nc.tensor.transpose(tk_lo[:, g * P:(g + 1) * P],
                                    krot[:, g * D + P:(g + 1) * D], ident)
            khi_d = kT_hi[:, :, ti * P:(ti + 1) * P]
            klo_d = kT_lo[:, :, ti * P:(ti + 1) * P]
            nc.vector.tensor_copy(khi_d,
                                  tk_hi.rearrange("p (g t) -> p g t", g=GPC))
            nc.scalar.copy(klo_d,
                           tk_lo.rearrange("p (g t) -> p g t", g=GPC))
            if use_fp8:
                nc.gpsimd.tensor_copy(k8[:, 0, :, ti * P:(ti + 1) * P],
                                      khi_d)
                nc.gpsimd.tensor_copy(k8[0:D - P, 1, :, ti * P:(ti + 1) * P],
                                      klo_d)

            # --- v copy (col HD is the ones column); SBUF-only -> Pool ---
            vb = GPC * D + GPC * HD
            nc.gpsimd.tensor_copy(
                v_sb[:, ti, :, 0:HD],
                natt[:, vb:vb + GPC * HD].rearrange("p (g c) -> p g c", g=GPC))

        COPYF = mybir.ActivationFunctionType.Copy

        def emit_proj_tile(ti):
            half = GPC // 2 * D  # 384
            natt = natp.tile([P, WCOLS], BF16, tag="natt", name="natt")
            pq_a = ps_proj.tile([P, half], F32, tag="pq_a", name="pq_a")
            pq_b = ps_proj.tile([P, half], F32, tag="pq_b", name="pq_b")
            pkv = ps_proj.tile([P, 2 * GPC * HD], F32, tag="pkv", name="pkv")
            if use_fp8proj:
                # P = 16*xh@Wh + xh@Wl + xl16@Wh = 512 * x@W (+O(1e-4))
                x8t = stream.tile([P, 3, 2, NE // 2, P], FP8, tag="x8t",
                                  name="x8t")
                nrow = 3 * NE * P  # 4608 contiguous elems/partition per ti
                nc.gpsimd.dma_start(
                    x8t.rearrange("p v pl ep t -> p (v pl ep t)"),
                    x8_d[:, ti * nrow:(ti + 1) * nrow])
                terms = ((x8t[:, 0], w8h), (x8t[:, 1], w8l),
                         (x8t[:, 2], w8h))
                for ep in range(NE // 2):
                    for t, (xv, wv) in enumerate(terms):
                        lhsT = xv[:, :, ep, :]
                        first = (ep == 0 and t == 0)
                        last = (ep == NE // 2 - 1 and t == 2)
                        nc.tensor.matmul(
                            pq_a, lhsT, wv[:, :, ep, 0:half],
                            start=first, stop=last, perf_mode=DR)
                        nc.tensor.matmul(
                            pq_b, lhsT, wv[:, :, ep, half:2 * half],
                            start=first, stop=last, perf_mode=DR)
                        nc.tensor.matmul(
                            pkv, lhsT, wv[:, :, ep, GPC * D:WCOLS],
                            start=first, stop=last, perf_mode=DR)
                sc = 1.0 / 512.0
                nc.scalar.activation(natt[:, 0:half], pq_a, COPYF, scale=sc)
                nc.scalar.activation(natt[:, half:2 * half], pq_b, COPYF,
                                     scale=sc)
                nc.vector.tensor_scalar_mul(natt[:, GPC * D:WCOLS], pkv, sc)
                return natt
            xti = stream.tile([P, NE, P], BF16, tag="xti", name="xti")
            nc.gpsimd.dma_start(
                xti, xt_d[:, ti * P:(ti + 1) * P].rearrange(
                    "(eo p) t -> p eo t", p=P))
            for eo in range(NE):
                lhsT = xti[:, eo, :]
                last = (eo == NE - 1) and not use_bias
                nc.tensor.matmul(
                    pq_a, lhsT, w_sb[:, eo, 0:half],
                    start=(eo == 0), stop=last)
                nc.tensor.matmul(
                    pq_b, lhsT, w_sb[:, eo, half:2 * half],
                    start=(eo == 0), stop=last)
                nc.tensor.matmul(
                    pkv, lhsT, w_sb[:, eo, GPC * D:WCOLS],
                    start=(eo == 0), stop=last)
            if use_bias:
                nc.tensor.matmul(pq_a, ones, b_sb[:, 0:half],
                                 start=False, stop=True)
                nc.tensor.matmul(pq_b, ones, b_sb[:, half:2 * half],
                                 start=False, stop=True)
                nc.tensor.matmul(pkv, ones, b_sb[:, GPC * D:WCOLS],
                                 start=False, stop=True)
            nc.scalar.copy(natt[:, 0:half], pq_a)
            nc.scalar.copy(natt[:, half:2 * half], pq_b)
            nc.vector.tensor_copy(natt[:, GPC * D:WCOLS], pkv)
            return natt

        # SDPA sub-chunks: (qoff, width).  qc0..2 are full 512 chunks; the
        # last 512 is split in two 256 halves so the first becomes ready
        # two proj tiles earlier (shrinks the post-proj tail).
        CHUNKS = [(qc * QCH, QCH) for qc in range(NQC)]
        BLOCKS = [(j, qo, w, kc) for (qo, w) in CHUNKS for j in range(GPC)
                  for kc in range((qo + w) // P)]
        LA = 8

        def sdpa_steps():
            """Generator: one SDPA block per next().  Groups interleave at
            sub-chunk granularity so only one (group, chunk) owns the
            o_ps/tpo rings at a time."""

            def emit_s(j, qo, w, kc):
                diag = kc * P >= qo
                off = max(kc * P - qo, 0)  # chunk-frame offset
                nq = w - off
                s_ps = ps_s.tile([P, QCH], F32, tag="sps", name="sps")
                sv = s_ps[:, off:w]
                if use_fp8 and (kc + 1) * P <= (qo // QCH) * QCH:
                    nc.tensor.matmul(
                        sv, k8[:, :, j, kc * P:(kc + 1) * P],
                        q8[:, :, j, qo + off:qo + off + nq],
                        start=True, stop=True, perf_mode=DR)
                else:
                    nc.tensor.matmul(
                        sv, kT_hi[:, j, kc * P:(kc + 1) * P],
                        qT_hi[:, j, qo + off:qo + off + nq],
                        start=True, stop=False)
                    nc.tensor.matmul(
                        sv, kT_lo[:, j, kc * P:(kc + 1) * P],
                        qT_lo[:, j, qo + off:qo + off + nq],
                        start=False, stop=True)
                pT = ppool.tile([P, QCH], BF16, tag="pT", name="pT")
                nc.scalar.activation(pT[:, off:w], sv,
                                     mybir.ActivationFunctionType.Exp,
                                     scale=SCALE)
                if diag:  # causal zeroing of the in-block triangle
                    nc.gpsimd.tensor_tensor(pT[:, off:off + P],
                                            pT[:, off:off + P],
                                            tri, mult)
                return pT

            pTs = {}
            for i in range(LA):
                pTs[BLOCKS[i]] = emit_s(*BLOCKS[i])
            o_ps = None
            for i, (j, qo, w, kc) in enumerate(BLOCKS):
                if i + LA < len(BLOCKS):
                    b = BLOCKS[i + LA]
                    pTs[b] = emit_s(*b)
                kmax = (qo + w) // P
                if kc == 0:
                    o_ps = ps_o.tile([HD + 1, QCH], F32, tag="ops",
                                     name="ops")
                pw = pTs.pop((j, qo, w, kc))
                off = max(kc * P - qo, 0)
                nc.tensor.matmul(o_ps[:, off:w], v_sb[:, kc, j, :],
                                 pw[:, off:w],
                                 start=(kc == 0), stop=(kc == kmax - 1))
                yield
                if kc != kmax - 1:
                    continue
                # ---- finalize (group j, chunk qo..qo+w) ----
                o_sb = opool.tile([HD + 1, QCH], F32, tag="o_sb",
                                  name="o_sb")
                nc.vector.tensor_copy(o_sb[:, 0:w], o_ps[:, 0:w])
                NB = w // P
                tpo = ps_o.tile([P, (QCH // P) * (HD + 1)], F32, tag="ops",
                                name="tpo")
                for blk in range(NB):
                    nc.tensor.transpose(
                        tpo[:, blk * (HD + 1):(blk + 1) * (HD + 1)],
                        o_sb[:, blk * P:(blk + 1) * P],
                        ident_f[:HD + 1, :HD + 1])
                nat = opool.tile([P, QCH // P, HD + 8], F32, tag="nat",
                                 name="nat")
                nc.vector.tensor_copy(
                    nat[:, 0:NB, 0:HD + 1],
                    tpo[:, 0:NB * (HD + 1)].rearrange(
                        "p (b c) -> p b c", b=NB))
                rec = opool.tile([P, QCH // P], F32, tag="rec", name="rec")
                nc.vector.reciprocal(rec[:, 0:NB], nat[:, 0:NB, HD])
                nc.vector.tensor_tensor(
                    nat[:, 0:NB, 0:HD], nat[:, 0:NB, 0:HD],
                    rec[:, 0:NB, None].to_broadcast((P, NB, HD)), mult)
                for blk in range(NB):
                    row0 = qo + blk * P
                    dst = out_d[row0:row0 + P,
                                j * D:(j + 1) * D].rearrange(
                        "t (r c) -> t r c", r=REP)
                    src_ap = nat[:, blk, None, 0:HD].to_broadcast(
                        (P, REP, HD))
                    nc.sync.dma_start(dst, src_ap)
                yield

        # Per-yield readiness: number of roped proj tiles required.
        def ready(b):
            _, qo, w, _ = b
            return (qo + w) // P

        reqs = []
        for i, b in enumerate(BLOCKS):
            j, qo, w, kc = b
            reqs.append(ready(BLOCKS[min(i + LA, len(BLOCKS) - 1)]))
            if kc == (qo + w) // P - 1:
                reqs.append(ready(b))

        gen = sdpa_steps()
        state = {"idx": 0}

        def drain(ti_done, budget):
            while budget > 0 and state["idx"] < len(reqs):
                if reqs[state["idx"]] > ti_done:
                    return
                next(gen)
                state["idx"] += 1
                budget -= 1

        pending = []
        for ti in range(NT):
            natt = emit_proj_tile(ti)
            pending.append((ti, natt))
            if len(pending) > 1:
                emit_rope(*pending.pop(0))
            drain(ti, 24)
        while pending:
            emit_rope(*pending.pop(0))
        drain(NT, 1 << 30)

    nc.compile()
    return nc


_NC_CACHE = {}


def _get_nc(use_bias=True):
    if use_bias not in _NC_CACHE:
        _NC_CACHE[use_bias] = _build_nc(use_bias)
    return _NC_CACHE[use_bias]


def _host_inputs(x, Wq, bq, Wk, bk, Wv, bv):
    j = np.arange(D // 2)
    angles = 1.0 / (THETA ** ((2.0 * j) / D))
    th = np.arange(T, dtype=np.float64)[:, None] * angles[None, :]
    cosn = np.cos(th).astype(ml_dtypes.bfloat16)
    sinn = np.sin(th).astype(ml_dtypes.bfloat16)

    perm_q = np.concatenate([np.arange(0, D, 2), np.arange(1, D, 2)])
    eo = np.concatenate([np.arange(0, HD, 2), np.arange(1, HD, 2)])

    Wq = np.asarray(Wq, np.float32)
    Wk = np.asarray(Wk, np.float32)
    Wv = np.asarray(Wv, np.float32)
    bq = np.asarray(bq, np.float32)
    bk = np.asarray(bk, np.float32)
    bv = np.asarray(bv, np.float32)
    x = np.asarray(x, np.float32)

    in_maps = []
    for c in range(8):
        b, gh = divmod(c, 2)
        gs = [gh * GPC + jj for jj in range(GPC)]
        wblocks, bblocks = [], []
        for g in gs:
            wblocks.append(Wq[:, g * D:(g + 1) * D][:, perm_q])
            bblocks.append(bq[g * D:(g + 1) * D][perm_q])
        for g in gs:
            wblocks.append(Wk[:, g * HD:(g + 1) * HD][:, eo])
            bblocks.append(bk[g * HD:(g + 1) * HD][eo])
        for g in gs:
            wblocks.append(Wv[:, g * HD:(g + 1) * HD])
            bblocks.append(bv[g * HD:(g + 1) * HD])
        w_core = np.ascontiguousarray(np.concatenate(wblocks, axis=1))
        b_core = np.concatenate(bblocks)[None, :].astype(ml_dtypes.bfloat16)
        b_core = np.ascontiguousarray(b_core)
        xt = np.ascontiguousarray(x[b].T)
        f8 = ml_dtypes.float8_e4m3
        xh = xt.astype(f8).astype(np.float32)
        w32 = 32.0 * w_core
        wh = w32.astype(f8).astype(np.float32)

        def xtile(a):
            # [E, T] -> [p, ti, pl, ep, tt]
            a = a.reshape(NE // 2, 2, P, NT, P)       # ep pl p ti tt
            return a.transpose(2, 3, 1, 0, 4)         # p ti pl ep tt

        x8 = np.stack([xtile((16.0 * xh).astype(f8)),
                       xtile(xh.astype(f8)),
                       xtile((16.0 * (xt - xh)).astype(f8))],
                      axis=2)                         # p ti v pl ep tt
        x8 = np.ascontiguousarray(x8.reshape(P, NT * 3 * NE * P))

        in_maps.append({
            "xt": xt.astype(ml_dtypes.bfloat16),
            "w": w_core.astype(ml_dtypes.bfloat16),
            "bias": b_core,
            "cos": cosn,
            "sin": sinn,
            "x8": x8,
            "w8h": wh.astype(f8),
            "w8l": (16.0 * (w32 - wh)).astype(f8),
            "z8": np.zeros((D - P, GPC * T), f8),
        })
    return in_maps


def kernel(x, Wq, bq, Wk, bk, Wv, bv, _trace=False, _trace_kwargs=None):
    in_maps = _host_inputs(x, Wq, bq, Wk, bk, Wv, bv)
    use_bias = bool(max(np.abs(np.asarray(b)).max() for b in (bq, bk, bv)) > 0)
    nc = _get_nc(use_bias)
    res = run_bass_kernel_spmd(nc, in_maps, core_ids=list(range(8)),
                               trace=_trace, **(_trace_kwargs or {}))
    out = np.empty((B, T, E), np.float32)
    for c in range(8):
        b, gh = divmod(c, 2)
        out[b, :, gh * GPC * D:(gh + 1) * GPC * D] = res.results[c]["out"]
    if _trace:
        return out, res
    return out
